# revision 14
# baseline (speedup 1.0000x reference)
"""DistMult edge scoring on 8 Trainium2 NeuronCores.

score[e] = sum_d node_emb[src[e], d] * rel_emb[e, d] * node_emb[dst[e], d]

Strategy (data-parallel over edges, per the sharding hint):
  - Edges (src, dst, rel_emb rows) are sharded evenly across the 8 cores;
    node_emb is replicated to every core's DRAM.
  - Per-edge head/tail rows are fetched with dma_gather (ANT gpsimd ucode).
    Its indices are int16, so edges are binned by (src//32768, dst//32768)
    into 16 bins; each bin gathers from a 32768-row window of the table
    with window-local indices.
  - Bins are padded to multiples of 128 and chopped into chunks of up to
    1024 edges; per chunk: gather head, gather tail, load rel, then
    head*tail*rel on DVE and an add-reduce over the hidden dim.
  - The edge permutation is undone on the host when unsharding.

Self-contained: imports only concourse + numpy; all shapes hardcoded.
"""

import numpy as np

from concourse import bacc, mybir
from concourse.bass_utils import run_bass_kernel_spmd
from concourse.tile import TileContext

N_NODES = 100000
N_EDGES = 150000
D = 512
P = 128
N_CORES = 8
EDGES_PER_CORE = N_EDGES // N_CORES      # 18750
# Two 65536-row windows: dma_gather indices are SIGNED int16 and interior
# negatives are used as (negative) row offsets from the base, so a base
# shifted +32768 rows covers rows [base-32768, base+32767].  Only TRAILING
# negative indices are trimmed by the ucode, so every chunk keeps >=16
# trailing zero-pad indices to stop the trim from eating real edges.
WIN_SPLIT = 50000                        # src/dst < split -> window 0
WIN_BASE = (32768, N_NODES - 32768)      # base rows 32768 / 67232
N_WINS = 2
N_BINS = N_WINS * N_WINS                 # 4
CHUNK_TILES = 8                          # max 128-edge tiles per dma_gather
CHUNK = CHUNK_TILES * P                  # 1024
CHUNK_VALID = CHUNK - 16                 # >=16 trailing pads per chunk
BUFS = 7


def plan_chunks(bin_counts):
    """bin_counts: per-bin max-over-cores edge counts (0 = skip).
    Returns (chunks, j_total, c_total); chunk = (bin_id, n_idx, valid, j0, c0).
    n_idx is a multiple of P; valid <= n_idx - 16 edges are real."""
    chunks = []
    j = 0  # tile-column offset into rel/score
    c = 0  # int16 column offset into the index tensors
    for b in range(len(bin_counts)):
        rem = int(bin_counts[b])
        while rem > 0:
            v = min(CHUNK_VALID, rem)
            n = min(CHUNK, (-(-(v + 16) // P)) * P)
            chunks.append((b, n, v, j, c))
            j += -(-n // P)
            c += n // 16
            rem -= v
    return chunks, j, c


def build_program(chunks, j_total, c_total, n_nodes=N_NODES, d=D, bufs=BUFS):
    """Build the single-core Bass program (same NEFF runs on all cores)."""
    f16 = mybir.dt.float16
    f32 = mybir.dt.float32
    nc = bacc.Bacc(None, target_bir_lowering=False, num_swdge_queues=4)
    node_emb = nc.declare_dram_parameter("node_emb", [n_nodes, d], f16, isOutput=False)
    rel = nc.declare_dram_parameter("rel", [P, j_total, d], f16, isOutput=False)
    srci = nc.declare_dram_parameter("srci", [P, c_total], mybir.dt.int16, isOutput=False)
    dsti = nc.declare_dram_parameter("dsti", [P, c_total], mybir.dt.int16, isOutput=False)
    score = nc.declare_dram_parameter("score", [P, j_total], f32, isOutput=True)

    with TileContext(nc) as tc:
        with (
            tc.tile_pool(name="const", bufs=1) as cpool,
            tc.tile_pool(name="emb", bufs=bufs) as epool,
        ):
            src_sb = cpool.tile([P, c_total], mybir.dt.int16, tag="srci")
            dst_sb = cpool.tile([P, c_total], mybir.dt.int16, tag="dsti")
            score_sb = cpool.tile([P, j_total], f32, tag="score")
            act_dump = cpool.tile([P, d], f16, tag="actdump")
            nc.sync.dma_start(out=src_sb[:], in_=srci[:])
            nc.sync.dma_start(out=dst_sb[:], in_=dsti[:])
            for ci, (b, n_idx, _v, j0, c0) in enumerate(chunks):
                a, bb = divmod(b, N_WINS)
                m = -(-n_idx // P)
                w = n_idx // 16
                head = epool.tile([P, CHUNK_TILES, d], f16, tag="head")
                tail = epool.tile([P, CHUNK_TILES, d], f16, tag="tail")
                relt = epool.tile([P, CHUNK_TILES, d], f16, tag="rel")
                # Spread gathers over the 4 SWDGE queues: each queue is
                # serviced by a different Q7 core pair and has its own
                # descriptor ring, so desc-gen and ring drain of adjacent
                # gathers can overlap.
                nc.gpsimd.dma_gather(
                    head[:, :m, :],
                    node_emb[WIN_BASE[a] :, :],
                    src_sb[:, c0 : c0 + w],
                    n_idx,
                    n_idx,
                    d,
                    queue_num=(2 * ci) % 4,
                )
                nc.gpsimd.dma_gather(
                    tail[:, :m, :],
                    node_emb[WIN_BASE[bb] :, :],
                    dst_sb[:, c0 : c0 + w],
                    n_idx,
                    n_idx,
                    d,
                    queue_num=(2 * ci + 1) % 4,
                )
                nc.sync.dma_start(out=relt[:, :m, :], in_=rel[:, j0 : j0 + m, :])
                # (tensor_tensor_reduce would fuse mult2+reduce, but
                # InstTensorTensorReduce hangs the device on this run path —
                # verified with a standalone single-core probe.)
                nc.vector.tensor_tensor(
                    out=head[:, :m, :], in0=head[:, :m, :], in1=tail[:, :m, :],
                    op=mybir.AluOpType.mult,
                )
                nc.vector.tensor_tensor(
                    out=head[:, :m, :], in0=head[:, :m, :], in1=relt[:, :m, :],
                    op=mybir.AluOpType.mult,
                )
                # Per-column add-reduce on the otherwise-idle Activation
                # engine (Copy + accum_out, f32 accumulator); frees DVE for
                # the two multiplies.
                for jj in range(m):
                    nc.scalar.activation(
                        out=act_dump[:, :],
                        in_=head[:, jj, :],
                        func=mybir.ActivationFunctionType.Copy,
                        accum_out=score_sb[:, j0 + jj : j0 + jj + 1],
                    )
            nc.sync.dma_start(out=score[:], in_=score_sb[:])
    # Run the Bacc compile pipeline (register allocation, event-semaphore
    # wait splitting) — the axon run path does not finalize for us.
    nc.finalize()
    return nc


def shard_and_plan(node_emb, rel_emb, src, dst, n_cores=N_CORES):
    """Contiguous equal edge shards + per-core binning into the 4
    (src-window, dst-window) bins; build in_maps + unshard positions.

    Returns (chunks, j_total, c_total, in_maps, positions) where positions =
    (pos_core, pos_p, pos_j) per global edge.
    """
    node_emb = np.ascontiguousarray(
        np.asarray(node_emb, dtype=np.float32).astype(np.float16)
    )
    rel_emb = np.asarray(rel_emb, dtype=np.float32).astype(np.float16)
    src64 = np.asarray(src).astype(np.int64)
    dst64 = np.asarray(dst).astype(np.int64)
    d = node_emb.shape[1]
    n_edges = len(src64)

    assert n_edges % n_cores == 0
    epc = n_edges // n_cores
    bins_g = (src64 >= WIN_SPLIT) * N_WINS + (dst64 >= WIN_SPLIT)
    core_bin_edges = [[None] * N_BINS for _ in range(n_cores)]
    counts = np.zeros((n_cores, N_BINS), np.int64)
    for c in range(n_cores):
        lo = c * epc
        eb = bins_g[lo : lo + epc]
        order = np.argsort(eb, kind="stable") + lo
        counts[c] = np.bincount(eb, minlength=N_BINS)
        start = np.zeros(N_BINS + 1, np.int64)
        start[1:] = np.cumsum(counts[c])
        for b in range(N_BINS):
            core_bin_edges[c][b] = order[start[b] : start[b + 1]]

    chunks, j_total, c_total = plan_chunks(counts.max(axis=0))

    pos_core = np.empty(n_edges, np.int8)
    pos_p = np.empty(n_edges, np.int32)
    pos_j = np.empty(n_edges, np.int32)
    in_maps = []
    for c in range(n_cores):
        src16 = np.zeros((P, c_total), np.int16)
        dst16 = np.zeros((P, c_total), np.int16)
        rel_t = np.zeros((P, j_total, d), np.float16)
        consumed = np.zeros(N_BINS, np.int64)
        for b, n_idx, valid, j0, c0 in chunks:
            e_all = core_bin_edges[c][b]
            e_chunk = e_all[consumed[b] : consumed[b] + valid]
            consumed[b] += valid
            nv = len(e_chunk)
            u = np.arange(n_idx)
            p, j = u % P, j0 + u // P
            li_s = np.zeros(n_idx, np.int16)
            li_d = np.zeros(n_idx, np.int16)
            if nv:
                a, bb = divmod(b, N_WINS)
                li_s[:nv] = (src64[e_chunk] - WIN_BASE[a]).astype(np.int16)
                li_d[:nv] = (dst64[e_chunk] - WIN_BASE[bb]).astype(np.int16)
                rel_t[p[:nv], j[:nv]] = rel_emb[e_chunk]
                pos_core[e_chunk] = c
                pos_p[e_chunk] = p[:nv]
                pos_j[e_chunk] = j[:nv]
            w = n_idx // 16
            src16[:, c0 : c0 + w] = np.tile(li_s.reshape(w, 16).T, (8, 1))
            dst16[:, c0 : c0 + w] = np.tile(li_d.reshape(w, 16).T, (8, 1))
        in_maps.append(
            {"node_emb": node_emb, "rel": rel_t, "srci": src16, "dsti": dst16}
        )
    return chunks, j_total, c_total, in_maps, (pos_core, pos_p, pos_j)


def _unshard(results, positions):
    pos_core, pos_p, pos_j = positions
    out = np.empty(len(pos_core), np.float32)
    for c in range(len(results)):
        m = pos_core == c
        sc = np.asarray(results[c]["score"])
        out[m] = sc[pos_p[m], pos_j[m]]
    return out


def _run(node_emb, rel_emb, src, dst, **spmd_kwargs):
    chunks, j_total, c_total, in_maps, positions = shard_and_plan(
        node_emb, rel_emb, src, dst
    )
    nc = build_program(chunks, j_total, c_total)
    res = run_bass_kernel_spmd(nc, in_maps, list(range(N_CORES)), **spmd_kwargs)
    return _unshard(res.results, positions), res


def kernel(node_emb, rel_emb, src, dst):
    out, _ = _run(node_emb, rel_emb, src, dst)
    return out


def _install_ntff_hook():
    """Provide antenv.axon_hooks (absent on this image) so bass_utils can
    NTFF-profile under axon, and skip the S3 artifact upload."""
    import contextlib
    import ctypes
    import sys
    import types

    from concourse import bass_utils as bu

    bu.upload_artifacts = lambda tmpdir: tmpdir  # no network in container

    if "antenv.axon_hooks" in sys.modules:
        return
    lib = ctypes.CDLL("/opt/axon/libaxon_pjrt.so")
    lib.axon_start_nrt_profile.argtypes = [
        ctypes.POINTER(ctypes.c_int64),
        ctypes.c_size_t,
    ]
    lib.axon_start_nrt_profile.restype = ctypes.c_int64
    lib.axon_stop_nrt_profile.argtypes = [ctypes.c_char_p]
    lib.axon_stop_nrt_profile.restype = ctypes.c_int64

    @contextlib.contextmanager
    def _hook(output_dir, device_ids):
        import jax

        jax.devices()
        if device_ids:
            ids = (ctypes.c_int64 * len(device_ids))(*device_ids)
            rc = lib.axon_start_nrt_profile(ids, len(device_ids))
        else:
            rc = lib.axon_start_nrt_profile(None, 0)
        if rc != 0:
            raise RuntimeError(f"axon_start_nrt_profile rc={rc}")
        try:
            yield
        finally:
            n = lib.axon_stop_nrt_profile(str(output_dir).encode())
            print(f"profile: {n} file(s) written to {output_dir}")

    mod = types.ModuleType("antenv.axon_hooks")
    mod.get_axon_ntff_profile_hook = lambda: _hook
    sys.modules["antenv.axon_hooks"] = mod


def kernel_profiled(node_emb, rel_emb, src, dst, trace_cores=None, tmpdir=None):
    """Like kernel() but also returns exec_time_ns from the NTFF profile."""
    _install_ntff_hook()
    out, res = _run(
        node_emb, rel_emb, src, dst,
        trace=True, trace_cores=trace_cores, tmpdir=tmpdir,
    )
    return out, res.exec_time_ns



# revision 15
# speedup vs baseline: 1.0516x; 1.0516x over previous
"""DistMult edge scoring on 8 Trainium2 NeuronCores.

score[e] = sum_d node_emb[src[e], d] * rel_emb[e, d] * node_emb[dst[e], d]

Strategy (data-parallel over edges, per the sharding hint):
  - Edges (src, dst, rel_emb rows) are sharded evenly across the 8 cores;
    node_emb is replicated to every core's DRAM.
  - Per-edge head/tail rows are fetched with dma_gather (ANT gpsimd ucode).
    Its indices are int16, so edges are binned by (src//32768, dst//32768)
    into 16 bins; each bin gathers from a 32768-row window of the table
    with window-local indices.
  - Bins are padded to multiples of 128 and chopped into chunks of up to
    1024 edges; per chunk: gather head, gather tail, load rel, then
    head*tail*rel on DVE and an add-reduce over the hidden dim.
  - The edge permutation is undone on the host when unsharding.

Self-contained: imports only concourse + numpy; all shapes hardcoded.
"""

import numpy as np

from concourse import bacc, mybir
from concourse.bass_utils import run_bass_kernel_spmd
from concourse.tile import TileContext

N_NODES = 100000
N_EDGES = 150000
D = 512
P = 128
N_CORES = 8
EDGES_PER_CORE = N_EDGES // N_CORES      # 18750
# Two 65536-row windows: dma_gather indices are SIGNED int16 and interior
# negatives are used as (negative) row offsets from the base, so a base
# shifted +32768 rows covers rows [base-32768, base+32767].  Only TRAILING
# negative indices are trimmed by the ucode, so every chunk keeps >=16
# trailing zero-pad indices to stop the trim from eating real edges.
WIN_SPLIT = 50000                        # src/dst < split -> window 0
WIN_BASE = (32768, N_NODES - 32768)      # base rows 32768 / 67232
N_WINS = 2
N_BINS = N_WINS * N_WINS                 # 4
CHUNK_TILES = 8                          # max 128-edge tiles per dma_gather
CHUNK = CHUNK_TILES * P                  # 1024
CHUNK_VALID = CHUNK - 16                 # >=16 trailing pads per chunk
BUFS = 7


def plan_chunks(bin_counts):
    """bin_counts: per-bin max-over-cores edge counts (0 = skip).
    Returns (chunks, j_total, c_total); chunk = (bin_id, n_idx, valid, j0, c0).
    n_idx is a multiple of P; valid <= n_idx - 16 edges are real."""
    chunks = []
    j = 0  # tile-column offset into rel/score
    c = 0  # int16 column offset into the index tensors
    for b in range(len(bin_counts)):
        rem = int(bin_counts[b])
        while rem > 0:
            v = min(CHUNK_VALID, rem)
            n = min(CHUNK, (-(-(v + 16) // P)) * P)
            chunks.append((b, n, v, j, c))
            j += -(-n // P)
            c += n // 16
            rem -= v
    return chunks, j, c


def build_program(chunks, j_total, c_total, n_nodes=N_NODES, d=D, bufs=BUFS):
    """Build the single-core Bass program (same NEFF runs on all cores)."""
    f16 = mybir.dt.float16
    f32 = mybir.dt.float32
    nc = bacc.Bacc(None, target_bir_lowering=False, num_swdge_queues=4)
    node_emb = nc.declare_dram_parameter("node_emb", [n_nodes, d], f16, isOutput=False)
    rel = nc.declare_dram_parameter("rel", [P, j_total, d], f16, isOutput=False)
    srci = nc.declare_dram_parameter("srci", [P, c_total], mybir.dt.int16, isOutput=False)
    dsti = nc.declare_dram_parameter("dsti", [P, c_total], mybir.dt.int16, isOutput=False)
    score = nc.declare_dram_parameter("score", [P, j_total], f32, isOutput=True)

    with TileContext(nc) as tc:
        with (
            tc.tile_pool(name="const", bufs=1) as cpool,
            tc.tile_pool(name="emb", bufs=bufs) as epool,
        ):
            src_sb = cpool.tile([P, c_total], mybir.dt.int16, tag="srci")
            dst_sb = cpool.tile([P, c_total], mybir.dt.int16, tag="dsti")
            score_sb = cpool.tile([P, j_total], f32, tag="score")
            act_dump = cpool.tile([P, d], f16, tag="actdump")
            nc.sync.dma_start(out=src_sb[:], in_=srci[:])
            nc.sync.dma_start(out=dst_sb[:], in_=dsti[:])
            for ci, (b, n_idx, _v, j0, c0) in enumerate(chunks):
                a, bb = divmod(b, N_WINS)
                m = -(-n_idx // P)
                w = n_idx // 16
                head = epool.tile([P, CHUNK_TILES, d], f16, tag="head")
                tail = epool.tile([P, CHUNK_TILES, d], f16, tag="tail")
                relt = epool.tile([P, CHUNK_TILES, d], f16, tag="rel")
                # Spread gathers over the 4 SWDGE queues: each queue is
                # serviced by a different Q7 core pair and has its own
                # descriptor ring, so desc-gen and ring drain of adjacent
                # gathers can overlap.
                nc.gpsimd.dma_gather(
                    head[:, :m, :],
                    node_emb[WIN_BASE[a] :, :],
                    src_sb[:, c0 : c0 + w],
                    n_idx,
                    n_idx,
                    d,
                    queue_num=(2 * ci) % 4,
                )
                nc.gpsimd.dma_gather(
                    tail[:, :m, :],
                    node_emb[WIN_BASE[bb] :, :],
                    dst_sb[:, c0 : c0 + w],
                    n_idx,
                    n_idx,
                    d,
                    queue_num=(2 * ci + 1) % 4,
                )
                nc.sync.dma_start(out=relt[:, :m, :], in_=rel[:, j0 : j0 + m, :])
                # (tensor_tensor_reduce would fuse mult2+reduce, but
                # InstTensorTensorReduce hangs the device on this run path —
                # verified with a standalone single-core probe.)
                nc.vector.tensor_tensor(
                    out=head[:, :m, :], in0=head[:, :m, :], in1=tail[:, :m, :],
                    op=mybir.AluOpType.mult,
                )
                nc.vector.tensor_tensor(
                    out=head[:, :m, :], in0=head[:, :m, :], in1=relt[:, :m, :],
                    op=mybir.AluOpType.mult,
                )
                # Split the add-reduce: a batched tensor_reduce on DVE for the
                # first columns, per-column Copy+accum on the Activation
                # engine for the rest (Act costs ~1.0us/col vs DVE ~0.53,
                # so Act takes the larger share only to offload DVE).
                k = min(3, m)
                nc.vector.tensor_reduce(
                    out=score_sb[:, j0 : j0 + k], in_=head[:, :k, :],
                    axis=mybir.AxisListType.X, op=mybir.AluOpType.add,
                )
                for jj in range(k, m):
                    nc.scalar.activation(
                        out=act_dump[:, :],
                        in_=head[:, jj, :],
                        func=mybir.ActivationFunctionType.Copy,
                        accum_out=score_sb[:, j0 + jj : j0 + jj + 1],
                    )
            nc.sync.dma_start(out=score[:], in_=score_sb[:])
    # Run the Bacc compile pipeline (register allocation, event-semaphore
    # wait splitting) — the axon run path does not finalize for us.
    nc.finalize()
    return nc


def shard_and_plan(node_emb, rel_emb, src, dst, n_cores=N_CORES):
    """Contiguous equal edge shards + per-core binning into the 4
    (src-window, dst-window) bins; build in_maps + unshard positions.

    Returns (chunks, j_total, c_total, in_maps, positions) where positions =
    (pos_core, pos_p, pos_j) per global edge.
    """
    node_emb = np.ascontiguousarray(
        np.asarray(node_emb, dtype=np.float32).astype(np.float16)
    )
    rel_emb = np.asarray(rel_emb, dtype=np.float32).astype(np.float16)
    src64 = np.asarray(src).astype(np.int64)
    dst64 = np.asarray(dst).astype(np.int64)
    d = node_emb.shape[1]
    n_edges = len(src64)

    assert n_edges % n_cores == 0
    epc = n_edges // n_cores
    bins_g = (src64 >= WIN_SPLIT) * N_WINS + (dst64 >= WIN_SPLIT)
    core_bin_edges = [[None] * N_BINS for _ in range(n_cores)]
    counts = np.zeros((n_cores, N_BINS), np.int64)
    for c in range(n_cores):
        lo = c * epc
        eb = bins_g[lo : lo + epc]
        order = np.argsort(eb, kind="stable") + lo
        counts[c] = np.bincount(eb, minlength=N_BINS)
        start = np.zeros(N_BINS + 1, np.int64)
        start[1:] = np.cumsum(counts[c])
        for b in range(N_BINS):
            core_bin_edges[c][b] = order[start[b] : start[b + 1]]

    chunks, j_total, c_total = plan_chunks(counts.max(axis=0))

    pos_core = np.empty(n_edges, np.int8)
    pos_p = np.empty(n_edges, np.int32)
    pos_j = np.empty(n_edges, np.int32)
    in_maps = []
    for c in range(n_cores):
        src16 = np.zeros((P, c_total), np.int16)
        dst16 = np.zeros((P, c_total), np.int16)
        rel_t = np.zeros((P, j_total, d), np.float16)
        consumed = np.zeros(N_BINS, np.int64)
        for b, n_idx, valid, j0, c0 in chunks:
            e_all = core_bin_edges[c][b]
            e_chunk = e_all[consumed[b] : consumed[b] + valid]
            consumed[b] += valid
            nv = len(e_chunk)
            u = np.arange(n_idx)
            p, j = u % P, j0 + u // P
            li_s = np.zeros(n_idx, np.int16)
            li_d = np.zeros(n_idx, np.int16)
            if nv:
                a, bb = divmod(b, N_WINS)
                li_s[:nv] = (src64[e_chunk] - WIN_BASE[a]).astype(np.int16)
                li_d[:nv] = (dst64[e_chunk] - WIN_BASE[bb]).astype(np.int16)
                rel_t[p[:nv], j[:nv]] = rel_emb[e_chunk]
                pos_core[e_chunk] = c
                pos_p[e_chunk] = p[:nv]
                pos_j[e_chunk] = j[:nv]
            w = n_idx // 16
            src16[:, c0 : c0 + w] = np.tile(li_s.reshape(w, 16).T, (8, 1))
            dst16[:, c0 : c0 + w] = np.tile(li_d.reshape(w, 16).T, (8, 1))
        in_maps.append(
            {"node_emb": node_emb, "rel": rel_t, "srci": src16, "dsti": dst16}
        )
    return chunks, j_total, c_total, in_maps, (pos_core, pos_p, pos_j)


def _unshard(results, positions):
    pos_core, pos_p, pos_j = positions
    out = np.empty(len(pos_core), np.float32)
    for c in range(len(results)):
        m = pos_core == c
        sc = np.asarray(results[c]["score"])
        out[m] = sc[pos_p[m], pos_j[m]]
    return out


def _run(node_emb, rel_emb, src, dst, **spmd_kwargs):
    chunks, j_total, c_total, in_maps, positions = shard_and_plan(
        node_emb, rel_emb, src, dst
    )
    nc = build_program(chunks, j_total, c_total)
    res = run_bass_kernel_spmd(nc, in_maps, list(range(N_CORES)), **spmd_kwargs)
    return _unshard(res.results, positions), res


def kernel(node_emb, rel_emb, src, dst):
    out, _ = _run(node_emb, rel_emb, src, dst)
    return out


def _install_ntff_hook():
    """Provide antenv.axon_hooks (absent on this image) so bass_utils can
    NTFF-profile under axon, and skip the S3 artifact upload."""
    import contextlib
    import ctypes
    import sys
    import types

    from concourse import bass_utils as bu

    bu.upload_artifacts = lambda tmpdir: tmpdir  # no network in container

    if "antenv.axon_hooks" in sys.modules:
        return
    lib = ctypes.CDLL("/opt/axon/libaxon_pjrt.so")
    lib.axon_start_nrt_profile.argtypes = [
        ctypes.POINTER(ctypes.c_int64),
        ctypes.c_size_t,
    ]
    lib.axon_start_nrt_profile.restype = ctypes.c_int64
    lib.axon_stop_nrt_profile.argtypes = [ctypes.c_char_p]
    lib.axon_stop_nrt_profile.restype = ctypes.c_int64

    @contextlib.contextmanager
    def _hook(output_dir, device_ids):
        import jax

        jax.devices()
        if device_ids:
            ids = (ctypes.c_int64 * len(device_ids))(*device_ids)
            rc = lib.axon_start_nrt_profile(ids, len(device_ids))
        else:
            rc = lib.axon_start_nrt_profile(None, 0)
        if rc != 0:
            raise RuntimeError(f"axon_start_nrt_profile rc={rc}")
        try:
            yield
        finally:
            n = lib.axon_stop_nrt_profile(str(output_dir).encode())
            print(f"profile: {n} file(s) written to {output_dir}")

    mod = types.ModuleType("antenv.axon_hooks")
    mod.get_axon_ntff_profile_hook = lambda: _hook
    sys.modules["antenv.axon_hooks"] = mod


def kernel_profiled(node_emb, rel_emb, src, dst, trace_cores=None, tmpdir=None):
    """Like kernel() but also returns exec_time_ns from the NTFF profile."""
    _install_ntff_hook()
    out, res = _run(
        node_emb, rel_emb, src, dst,
        trace=True, trace_cores=trace_cores, tmpdir=tmpdir,
    )
    return out, res.exec_time_ns

